# revision 21
# baseline (speedup 1.0000x reference)
"""Trainium2 Bass kernel for nn_MaxTimesPlusOpeningLiftingP4.

Computation (per rotation i of 4):
  ero[u,c,f]  = min_p (x[u+d_p, c] - ke_i[p,c,f]) * inva_i[p,c,f]
  res[u,f]    = sum_c max_p (tk_i[p,c,f] * ero_pad[u+d_p, c, f] + k_i[p,c,f])
with SAME zero padding on both x and ero, 5x5 window (P=25).

Key structure (v2):
  Rotation sharing in stage 1: rot90 of the kernels only PERMUTES which
  affine coefficient pairs with which spatial offset, so the 25 affine
  planes A_q(u) = x(u)*a_q + b_q are computed ONCE (ACT engine, over the
  full padded extent so x's zero padding automatically yields the correct
  boundary term b_q) and each rotation's min-chain consumes plane
  q = perm_i[p] at shift p.  This cuts stage-1 affine work 4x.
  Stage 2 is per-rotation (input ero_i differs): 24 affine terms (mostly
  ACT, a few on DVE for balance) + 24 DVE tensor_tensor(max) merges.
  c-sum via TensorE matmul with a 0/1 selection matrix -> PSUM, ACT copy
  -> SBUF, DMA out.  Halo exchange + zero fixups via SBUF-SBUF DMAs.

Device layout: 120 SBUF partitions = (c=3, f=8, j=5 row-chunks); pixels on
the free dim in padded per-chunk buffers of 30 rows x (2 img x 132 cols),
images BLOCK-interleaved per row so every 5x5 shift is a flat AP offset
and per-image sub-views stay packed (2x/4x DVE modes).
Sharding: pure data parallel, 2 images per core on 8 cores.  Host does
weight prep, x replication, and output reassembly (host work is not on
the device clock).
"""
import numpy as np

EPS = 1e-7
B, H, W, C = 16, 128, 128, 3
KH, KW, F = 5, 5, 8
P = KH * KW
NJ = 5
ROWS = [26, 26, 26, 26, 24]
CH_START = [0, 26, 52, 78, 104]
RB = 30
WP = 132
NPART = 120           # (c,f,j): partition = (c*8+f)*5 + j
NCORES = 8
BPC = B // NCORES     # images per core (block-interleaved per row)
IL = BPC
WPB = WP * IL         # padded row in elements (264)
FDB = 26 * 128 * IL   # interior free size per op (6656)
USE_FP16 = True
S2_DVE = 4            # of the 24 stage-2 affine terms, how many on DVE
SKEW = 2              # stage-1 merge lag (in planes) between rotations

_CACHE = {}


def _part(c, f, j):
    return (c * 8 + f) * 5 + j


def _make_weights(kernel, timesKernel):
    """[120, 400] f32; col = (rot*25+p)*4 + slot, slot 0=a 1=b 2=tk 3=k."""
    kernel = np.asarray(kernel, np.float32)
    timesKernel = np.asarray(timesKernel, np.float32)
    k_ero = kernel[::-1, ::-1]
    t_ero = timesKernel[::-1, ::-1]
    Wt = np.zeros((NPART, 4 * P * 4), np.float32)
    for i in range(4):
        k_rot = np.rot90(kernel, k=i, axes=(0, 1)).reshape(P, C, F)
        tk_rot = np.rot90(timesKernel, k=i, axes=(0, 1)).reshape(P, C, F)
        ke_rot = np.rot90(k_ero, k=i, axes=(0, 1)).reshape(P, C, F)
        tke_rot = np.rot90(t_ero, k=i, axes=(0, 1)).reshape(P, C, F)
        a = (1.0 / (tke_rot.astype(np.float64) + EPS)).astype(np.float32)
        b = (-ke_rot * a).astype(np.float32)
        for c in range(C):
            for f in range(F):
                pi = (c * 8 + f) * 5 + np.arange(NJ)
                for p in range(P):
                    col = (i * P + p) * 4
                    Wt[pi, col + 0] = a[p, c, f]
                    Wt[pi, col + 1] = b[p, c, f]
                    Wt[pi, col + 2] = tk_rot[p, c, f]
                    Wt[pi, col + 3] = k_rot[p, c, f]
    return Wt


def _make_csum():
    S = np.zeros((NPART, 40), np.float32)
    for c in range(C):
        for f in range(F):
            for j in range(NJ):
                S[_part(c, f, j), f * NJ + j] = 1.0
    return S


def _build_xrep(x):
    """x [BPC,H,W,C] -> [120, RB*WPB] padded chunks, img block-interleaved."""
    xpad = np.zeros((BPC, H + 6, W + 4, C), np.float32)
    xpad[:, 2:2 + H, 2:2 + W, :] = x
    xr = np.zeros((NPART, RB, IL, WP), np.float32)
    for c in range(C):
        for j in range(NJ):
            blk = xpad[:, CH_START[j]:CH_START[j] + RB, :, c]  # [IL,RB,WP]
            blk = np.moveaxis(blk, 0, 1)                       # [RB,IL,WP]
            for f in range(F):
                xr[(c * 8 + f) * 5 + j] = blk
    return xr.reshape(NPART, RB * IL * WP)


def _rot_perms():
    """perm[i][p] = base coeff index used by rotation i at patch pos p."""
    idx = np.arange(P).reshape(KH, KW)
    return [np.rot90(idx, k=i).reshape(P).copy() for i in range(4)]


def _build_program():
    import concourse.bass as bass
    import concourse.bacc as bacc
    import concourse.mybir as mybir
    import concourse.tile as tile

    f32 = mybir.dt.float32
    dt_c = mybir.dt.float16 if USE_FP16 else mybir.dt.float32
    Alu = mybir.AluOpType
    Act = mybir.ActivationFunctionType

    perms = _rot_perms()

    nc = bacc.Bacc("TRN2", target_bir_lowering=False, debug=False,
                   num_devices=NCORES)
    x_in = nc.dram_tensor("x8", [NPART, RB * WPB], dt_c, kind="ExternalInput")
    wts_in = nc.dram_tensor("wts", [NPART, 4 * P * 4], f32, kind="ExternalInput")
    cs_in = nc.dram_tensor("csum", [NPART, 40], dt_c, kind="ExternalInput")
    out_d = nc.dram_tensor("out", [4, 40, FDB], dt_c, kind="ExternalOutput")

    def sb_ap(t, part_off, free_off, dims):
        row = t.shape[1]
        ap = [[sp * row, cnt] for sp, cnt in dims[0]]
        ap += [[se, cnt] for se, cnt in dims[1]]
        return bass.AP(tensor=t.tensor,
                       offset=t.offset + part_off * row + free_off, ap=ap)

    def shift_view(t, p):
        """interior-shaped view of padded tile t at 5x5 shift index p."""
        return sb_ap(t, 0, (p // 5) * WPB + (p % 5),
                     [[(1, NPART)], [(WPB, 26), (WP, IL), (1, 128)]])

    def interior(t):
        return shift_view(t, 12)

    NCHUNK = 4            # c-sum PSUM chunking
    CHW = FDB // NCHUNK   # 1664 elems -> 4-bank PSUM tiles

    with tile.TileContext(nc) as tc:
        with (
            tc.tile_pool(name="singles", bufs=1) as singles,
            tc.tile_pool(name="psum", bufs=2, space="PSUM") as psum_pool,
        ):
            wts = singles.tile([NPART, 4 * P * 4], f32, tag="wts")
            cs = singles.tile([NPART, 40], dt_c, tag="cs")
            zeros = singles.tile([NPART, 2 * WPB], dt_c, tag="zeros")
            nc.scalar.dma_start(out=wts[:], in_=wts_in[:])
            nc.scalar.dma_start(out=cs[:], in_=cs_in[:])
            nc.gpsimd.memset(zeros[:], 0.0)

            accs = [singles.tile([NPART, RB * WPB], dt_c, tag=f"ero{k}",
                                 name=f"ero{k}") for k in range(4)]
            # only the PAD regions need zeroing (interior is overwritten):
            # rows 0-1, rows 28-29, and W-pad cols of interior rows
            for t in accs:
                nc.gpsimd.memset(sb_ap(t, 0, 0,
                                       [[(1, NPART)], [(1, 2 * WPB)]]), 0.0)
                nc.gpsimd.memset(sb_ap(t, 0, 28 * WPB,
                                       [[(1, NPART)], [(1, 2 * WPB)]]), 0.0)
                nc.gpsimd.memset(sb_ap(t, 0, 2 * WPB,
                                       [[(1, NPART)],
                                        [(WPB, 26), (WP, IL), (130, 2),
                                         (1, 2)]]), 0.0)

            def wcol(rot, p, slot):
                return (rot * P + p) * 4 + slot

            def scl(rot, p, slot):
                c0 = wcol(rot, p, slot)
                return wts[:, c0:c0 + 1]

            def fixups(ero):
                """zero garbage rows + halo exchange for one erosion buf."""
                nc.sync.dma_start(
                    out=sb_ap(ero, 4, 26 * WPB, [[(5, 24)], [(1, 2 * WPB)]]),
                    in_=sb_ap(zeros, 4, 0, [[(5, 24)], [(1, 2 * WPB)]]))
                for jj in range(1, NJ):
                    nc.sync.dma_start(
                        out=sb_ap(ero, jj, 0, [[(5, 24)], [(1, 2 * WPB)]]),
                        in_=sb_ap(ero, jj - 1, 26 * WPB,
                                  [[(5, 24)], [(1, 2 * WPB)]]))
                for jj in range(NJ - 1):
                    nc.sync.dma_start(
                        out=sb_ap(ero, jj, 28 * WPB,
                                  [[(5, 24)], [(1, 2 * WPB)]]),
                        in_=sb_ap(ero, jj + 1, 2 * WPB,
                                  [[(5, 24)], [(1, 2 * WPB)]]))

            # ---------------- stage 1: shared-plane erosion ----------------
            # Rotation i's merges lag i*SKEW planes behind plane production,
            # so rotation 0's erosion completes early and its stage-2 ACT
            # work can pre-execute while later rotations still merge.
            with tc.tile_pool(name="s1", bufs=1) as s1_singles, \
                 tc.tile_pool(name="planes", bufs=7) as plane_pool:
                xr = s1_singles.tile([NPART, RB * WPB], dt_c, tag="xrep",
                                     name="xrep")
                nc.sync.dma_start(out=xr[:], in_=x_in[:])

                # init: center term q=12 (perm_i[12]==12 for all i)
                for i in range(4):
                    nc.vector.tensor_scalar(
                        out=interior(accs[i]), in0=shift_view(xr, 12),
                        scalar1=scl(0, 12, 0), scalar2=scl(0, 12, 1),
                        op0=Alu.mult, op1=Alu.add)

                inv = [np.argsort(pm) for pm in perms]
                QL = [qq for qq in range(P) if qq != 12]
                planes = {}
                NQ = len(QL)
                for t in range(NQ + 3 * SKEW):
                    if t < NQ:
                        q = QL[t]
                        A = plane_pool.tile([NPART, RB * WPB], dt_c,
                                            tag="plane", name="plane")
                        nc.scalar.activation(
                            out=A[:], in_=xr[:], func=Act.Identity,
                            bias=scl(0, q, 1), scale=scl(0, q, 0))
                        planes[t] = A
                    for i in range(4):
                        tt = t - i * SKEW
                        if 0 <= tt < NQ:
                            q = QL[tt]
                            p_i = int(inv[i][q])
                            nc.vector.tensor_tensor(
                                out=interior(accs[i]),
                                in0=shift_view(planes[tt], p_i),
                                in1=interior(accs[i]), op=Alu.min)
                            if tt == NQ - 1:
                                fixups(accs[i])

            # ---------------- stage 2: dilation + c-sum ----------------
            # start with dh=0 terms so the first ops depend only on the
            # garbage-zero DMA, not the halo DMAs
            ORDER2 = ([12, 10, 11, 13, 14] + list(range(5, 10))
                      + list(range(15, 20)) + list(range(0, 5))
                      + list(range(20, 25)))

            with (
                tc.tile_pool(name="tmp", bufs=5) as tmp_pool,
                tc.tile_pool(name="acc2", bufs=3) as acc2_pool,
                tc.tile_pool(name="res", bufs=2) as res_pool,
            ):
                def csum_out(rot, acc2):
                    res = res_pool.tile([40, FDB], dt_c, tag="res", name="res")
                    for h in range(NCHUNK):
                        ps = psum_pool.tile([40, CHW], f32, tag="ps", name="ps")
                        base = h * CHW
                        for k in range((CHW + 511) // 512):
                            n0 = k * 512
                            n1 = min(CHW, n0 + 512)
                            nc.tensor.matmul(ps[:, n0:n1], cs[:, 0:40],
                                             acc2[:, base + n0:base + n1],
                                             start=True, stop=True)
                        nc.scalar.copy(res[:, base:base + CHW], ps[:])
                    nc.sync.dma_start(out=out_d[rot], in_=res[:])

                pending = None
                for rot in range(4):
                    ero = accs[rot]
                    acc2 = acc2_pool.tile([NPART, FDB], dt_c, tag="acc2",
                                          name="acc2")
                    p0 = ORDER2[0]
                    nc.vector.tensor_scalar(
                        out=acc2[:], in0=shift_view(ero, p0),
                        scalar1=scl(rot, p0, 2), scalar2=scl(rot, p0, 3),
                        op0=Alu.mult, op1=Alu.add)
                    for i_p, p in enumerate(ORDER2[1:]):
                        t = tmp_pool.tile([NPART, FDB], dt_c, tag="tmp",
                                          name="tmp")
                        if (i_p * S2_DVE) % 24 < S2_DVE:
                            nc.vector.tensor_scalar(
                                out=t[:], in0=shift_view(ero, p),
                                scalar1=scl(rot, p, 2), scalar2=scl(rot, p, 3),
                                op0=Alu.mult, op1=Alu.add)
                        else:
                            nc.scalar.activation(
                                out=t[:], in_=shift_view(ero, p),
                                func=Act.Identity, bias=scl(rot, p, 3),
                                scale=scl(rot, p, 2))
                        nc.vector.tensor_tensor(out=acc2[:], in0=t[:],
                                                in1=acc2[:], op=Alu.max)
                    if pending is not None:
                        csum_out(*pending)
                    pending = (rot, acc2)
                csum_out(*pending)
    nc.compile()
    return nc


def _get_program():
    if "nc" not in _CACHE:
        _CACHE["nc"] = _build_program()
    return _CACHE["nc"]


def kernel(x, kernel, timesKernel):
    x = np.ascontiguousarray(np.asarray(x, np.float32))
    Wt = _make_weights(kernel, timesKernel)
    S = _make_csum()

    nc = _get_program()
    from concourse.bass_utils import run_bass_kernel_spmd
    dt_np = np.float16 if USE_FP16 else np.float32
    in_maps = []
    for i in range(NCORES):
        xrh = _build_xrep(x[i * BPC:(i + 1) * BPC]).astype(dt_np)
        in_maps.append({"x8": xrh, "wts": Wt, "csum": S.astype(dt_np)})

    import os
    trace = os.environ.get("BASS_TRACE", "0") == "1"
    r = run_bass_kernel_spmd(nc, in_maps, core_ids=list(range(NCORES)),
                             trace=trace)
    _CACHE["last_results"] = r
    outs = [m["out"] for m in r.results]

    full = np.empty((B, 4, H, W, F), np.float32)
    for i in range(NCORES):
        O = outs[i].astype(np.float32).reshape(4, 40, 26, IL, 128)
        for rot in range(4):
            for f in range(F):
                for j in range(NJ):
                    rws = ROWS[j]
                    for bb in range(BPC):
                        full[i * BPC + bb, rot,
                             CH_START[j]:CH_START[j] + rws, :, f] = \
                            O[rot, f * NJ + j, :rws, bb, :]
    return full


# revision 24
# speedup vs baseline: 1.0779x; 1.0779x over previous
"""Trainium2 Bass kernel for nn_MaxTimesPlusOpeningLiftingP4.

Computation (per rotation i of 4):
  ero[u,c,f]  = min_p (x[u+d_p, c] - ke_i[p,c,f]) * inva_i[p,c,f]
  res[u,f]    = sum_c max_p (tk_i[p,c,f] * ero_pad[u+d_p, c, f] + k_i[p,c,f])
with SAME zero padding on both x and ero, 5x5 window (P=25).

Key structure (v2):
  Rotation sharing in stage 1: rot90 of the kernels only PERMUTES which
  affine coefficient pairs with which spatial offset, so the 25 affine
  planes A_q(u) = x(u)*a_q + b_q are computed ONCE (ACT engine, over the
  full padded extent so x's zero padding automatically yields the correct
  boundary term b_q) and each rotation's min-chain consumes plane
  q = perm_i[p] at shift p.  This cuts stage-1 affine work 4x.
  Stage 2 is per-rotation (input ero_i differs): 24 affine terms (mostly
  ACT, a few on DVE for balance) + 24 DVE tensor_tensor(max) merges.
  c-sum via TensorE matmul with a 0/1 selection matrix -> PSUM, ACT copy
  -> SBUF, DMA out.  Halo exchange + zero fixups via SBUF-SBUF DMAs.

Device layout: 120 SBUF partitions = (c=3, f=8, j=5 row-chunks); pixels on
the free dim in padded per-chunk buffers of 30 rows x (2 img x 132 cols),
images BLOCK-interleaved per row so every 5x5 shift is a flat AP offset
and per-image sub-views stay packed (2x/4x DVE modes).
Sharding: pure data parallel, 2 images per core on 8 cores.  Host does
weight prep, x replication, and output reassembly (host work is not on
the device clock).
"""
import numpy as np

EPS = 1e-7
B, H, W, C = 16, 128, 128, 3
KH, KW, F = 5, 5, 8
P = KH * KW
NJ = 5
ROWS = [26, 26, 26, 26, 24]
CH_START = [0, 26, 52, 78, 104]
RB = 30
WP = 132
NPART = 120           # (c,f,j): partition = (c*8+f)*5 + j
NCORES = 8
BPC = B // NCORES     # images per core (block-interleaved per row)
IL = BPC
WPB = WP * IL         # padded row in elements (264)
FDB = 26 * 128 * IL   # interior free size per op (6656)
USE_FP16 = True
S2_DVE = 7            # of the 24 stage-2 affine terms, how many on DVE
SKEW = 0              # stage-1 merge lag (in planes) between rotations

_CACHE = {}


def _part(c, f, j):
    return (c * 8 + f) * 5 + j


def _make_weights(kernel, timesKernel):
    """[120, 400] f32; col = (rot*25+p)*4 + slot, slot 0=a 1=b 2=tk 3=k."""
    kernel = np.asarray(kernel, np.float32)
    timesKernel = np.asarray(timesKernel, np.float32)
    k_ero = kernel[::-1, ::-1]
    t_ero = timesKernel[::-1, ::-1]
    Wt = np.zeros((NPART, 4 * P * 4), np.float32)
    for i in range(4):
        k_rot = np.rot90(kernel, k=i, axes=(0, 1)).reshape(P, C, F)
        tk_rot = np.rot90(timesKernel, k=i, axes=(0, 1)).reshape(P, C, F)
        ke_rot = np.rot90(k_ero, k=i, axes=(0, 1)).reshape(P, C, F)
        tke_rot = np.rot90(t_ero, k=i, axes=(0, 1)).reshape(P, C, F)
        a = (1.0 / (tke_rot.astype(np.float64) + EPS)).astype(np.float32)
        b = (-ke_rot * a).astype(np.float32)
        for c in range(C):
            for f in range(F):
                pi = (c * 8 + f) * 5 + np.arange(NJ)
                for p in range(P):
                    col = (i * P + p) * 4
                    Wt[pi, col + 0] = a[p, c, f]
                    Wt[pi, col + 1] = b[p, c, f]
                    Wt[pi, col + 2] = tk_rot[p, c, f]
                    Wt[pi, col + 3] = k_rot[p, c, f]
    return Wt


def _make_csum():
    S = np.zeros((NPART, 40), np.float32)
    for c in range(C):
        for f in range(F):
            for j in range(NJ):
                S[_part(c, f, j), f * NJ + j] = 1.0
    return S


def _build_xrep(x):
    """x [BPC,H,W,C] -> [120, RB*WPB] padded chunks, img block-interleaved."""
    xpad = np.zeros((BPC, H + 6, W + 4, C), np.float32)
    xpad[:, 2:2 + H, 2:2 + W, :] = x
    xr = np.zeros((NPART, RB, IL, WP), np.float32)
    for c in range(C):
        for j in range(NJ):
            blk = xpad[:, CH_START[j]:CH_START[j] + RB, :, c]  # [IL,RB,WP]
            blk = np.moveaxis(blk, 0, 1)                       # [RB,IL,WP]
            for f in range(F):
                xr[(c * 8 + f) * 5 + j] = blk
    return xr.reshape(NPART, RB * IL * WP)


def _rot_perms():
    """perm[i][p] = base coeff index used by rotation i at patch pos p."""
    idx = np.arange(P).reshape(KH, KW)
    return [np.rot90(idx, k=i).reshape(P).copy() for i in range(4)]


def _build_program():
    import concourse.bass as bass
    import concourse.bacc as bacc
    import concourse.mybir as mybir
    import concourse.tile as tile

    f32 = mybir.dt.float32
    dt_c = mybir.dt.float16 if USE_FP16 else mybir.dt.float32
    Alu = mybir.AluOpType
    Act = mybir.ActivationFunctionType

    perms = _rot_perms()

    nc = bacc.Bacc("TRN2", target_bir_lowering=False, debug=False,
                   num_devices=NCORES)
    x_in = nc.dram_tensor("x8", [NPART, RB * WPB], dt_c, kind="ExternalInput")
    wts_in = nc.dram_tensor("wts", [NPART, 4 * P * 4], f32, kind="ExternalInput")
    cs_in = nc.dram_tensor("csum", [NPART, 40], dt_c, kind="ExternalInput")
    out_d = nc.dram_tensor("out", [4, 40, FDB], dt_c, kind="ExternalOutput")

    def sb_ap(t, part_off, free_off, dims):
        row = t.shape[1]
        ap = [[sp * row, cnt] for sp, cnt in dims[0]]
        ap += [[se, cnt] for se, cnt in dims[1]]
        return bass.AP(tensor=t.tensor,
                       offset=t.offset + part_off * row + free_off, ap=ap)

    def shift_view(t, p):
        """interior-shaped view of padded tile t at 5x5 shift index p."""
        return sb_ap(t, 0, (p // 5) * WPB + (p % 5),
                     [[(1, NPART)], [(WPB, 26), (WP, IL), (1, 128)]])

    def interior(t):
        return shift_view(t, 12)

    NCHUNK = 4            # c-sum PSUM chunking
    CHW = FDB // NCHUNK   # 1664 elems -> 4-bank PSUM tiles

    with tile.TileContext(nc) as tc:
        with (
            tc.tile_pool(name="singles", bufs=1) as singles,
            tc.tile_pool(name="psum", bufs=2, space="PSUM") as psum_pool,
        ):
            wts = singles.tile([NPART, 4 * P * 4], f32, tag="wts")
            cs = singles.tile([NPART, 40], dt_c, tag="cs")
            zeros = singles.tile([NPART, 2 * WPB], dt_c, tag="zeros")
            nc.scalar.dma_start(out=wts[:], in_=wts_in[:])
            nc.scalar.dma_start(out=cs[:], in_=cs_in[:])
            nc.gpsimd.memset(zeros[:], 0.0)

            accs = [singles.tile([NPART, RB * WPB], dt_c, tag=f"ero{k}",
                                 name=f"ero{k}") for k in range(4)]
            # only the PAD regions need zeroing (interior is overwritten):
            # rows 0-1, rows 28-29, and W-pad cols of interior rows
            for t in accs:
                nc.gpsimd.memset(sb_ap(t, 0, 0,
                                       [[(1, NPART)], [(1, 2 * WPB)]]), 0.0)
                nc.gpsimd.memset(sb_ap(t, 0, 28 * WPB,
                                       [[(1, NPART)], [(1, 2 * WPB)]]), 0.0)
                nc.gpsimd.memset(sb_ap(t, 0, 2 * WPB,
                                       [[(1, NPART)],
                                        [(WPB, 26), (WP, IL), (130, 2),
                                         (1, 2)]]), 0.0)

            def wcol(rot, p, slot):
                return (rot * P + p) * 4 + slot

            def scl(rot, p, slot):
                c0 = wcol(rot, p, slot)
                return wts[:, c0:c0 + 1]

            def fixups(ero):
                """zero garbage rows + halo exchange for one erosion buf."""
                nc.sync.dma_start(
                    out=sb_ap(ero, 4, 26 * WPB, [[(5, 24)], [(1, 2 * WPB)]]),
                    in_=sb_ap(zeros, 4, 0, [[(5, 24)], [(1, 2 * WPB)]]))
                for jj in range(1, NJ):
                    nc.sync.dma_start(
                        out=sb_ap(ero, jj, 0, [[(5, 24)], [(1, 2 * WPB)]]),
                        in_=sb_ap(ero, jj - 1, 26 * WPB,
                                  [[(5, 24)], [(1, 2 * WPB)]]))
                for jj in range(NJ - 1):
                    nc.sync.dma_start(
                        out=sb_ap(ero, jj, 28 * WPB,
                                  [[(5, 24)], [(1, 2 * WPB)]]),
                        in_=sb_ap(ero, jj + 1, 2 * WPB,
                                  [[(5, 24)], [(1, 2 * WPB)]]))

            # ---------------- stage 1: shared-plane erosion ----------------
            # All pools are opened CONCURRENTLY with stage 2's pools (below)
            # so no SBUF region is shared between the phases: SBUF reuse
            # would impose a write-after-read barrier that serializes stage 2
            # behind the very last stage-1 merge.
            with tc.tile_pool(name="s1", bufs=1) as s1_singles, \
                 tc.tile_pool(name="planes", bufs=3) as plane_pool, \
                 tc.tile_pool(name="tmp", bufs=3) as tmp_pool, \
                 tc.tile_pool(name="acc2", bufs=2) as acc2_pool, \
                 tc.tile_pool(name="res", bufs=1) as res_pool:
                xr = s1_singles.tile([NPART, RB * WPB], dt_c, tag="xrep",
                                     name="xrep")
                nc.sync.dma_start(out=xr[:], in_=x_in[:])

                # init: center term q=12 (perm_i[12]==12 for all i)
                for i in range(4):
                    nc.vector.tensor_scalar(
                        out=interior(accs[i]), in0=shift_view(xr, 12),
                        scalar1=scl(0, 12, 0), scalar2=scl(0, 12, 1),
                        op0=Alu.mult, op1=Alu.add)

                inv = [np.argsort(pm) for pm in perms]
                QL = [qq for qq in range(P) if qq != 12]
                planes = {}
                NQ = len(QL)
                for t in range(NQ + 3 * SKEW):
                    if t < NQ:
                        q = QL[t]
                        A = plane_pool.tile([NPART, RB * WPB], dt_c,
                                            tag="plane", name="plane")
                        nc.scalar.activation(
                            out=A[:], in_=xr[:], func=Act.Identity,
                            bias=scl(0, q, 1), scale=scl(0, q, 0))
                        planes[t] = A
                    for i in range(4):
                        tt = t - i * SKEW
                        if 0 <= tt < NQ:
                            q = QL[tt]
                            p_i = int(inv[i][q])
                            nc.vector.tensor_tensor(
                                out=interior(accs[i]),
                                in0=shift_view(planes[tt], p_i),
                                in1=interior(accs[i]), op=Alu.min)
                            if tt == NQ - 1:
                                fixups(accs[i])

                # ---------------- stage 2: dilation + c-sum ----------------
                # start with dh=0 terms so the first ops depend only on the
                # garbage-zero DMA, not the halo DMAs
                ORDER2 = ([12, 10, 11, 13, 14] + list(range(5, 10))
                          + list(range(15, 20)) + list(range(0, 5))
                          + list(range(20, 25)))

                def csum_out(rot, acc2):
                    res = res_pool.tile([40, FDB], dt_c, tag="res", name="res")
                    for h in range(NCHUNK):
                        ps = psum_pool.tile([40, CHW], f32, tag="ps", name="ps")
                        base = h * CHW
                        for k in range((CHW + 511) // 512):
                            n0 = k * 512
                            n1 = min(CHW, n0 + 512)
                            nc.tensor.matmul(ps[:, n0:n1], cs[:, 0:40],
                                             acc2[:, base + n0:base + n1],
                                             start=True, stop=True)
                        nc.scalar.copy(res[:, base:base + CHW], ps[:])
                    nc.sync.dma_start(out=out_d[rot], in_=res[:])

                pending = None
                for rot in range(4):
                    ero = accs[rot]
                    acc2 = acc2_pool.tile([NPART, FDB], dt_c, tag="acc2",
                                          name="acc2")
                    p0 = ORDER2[0]
                    nc.vector.tensor_scalar(
                        out=acc2[:], in0=shift_view(ero, p0),
                        scalar1=scl(rot, p0, 2), scalar2=scl(rot, p0, 3),
                        op0=Alu.mult, op1=Alu.add)
                    for i_p, p in enumerate(ORDER2[1:]):
                        t = tmp_pool.tile([NPART, FDB], dt_c, tag="tmp",
                                          name="tmp")
                        if (i_p * S2_DVE) % 24 < S2_DVE:
                            nc.vector.tensor_scalar(
                                out=t[:], in0=shift_view(ero, p),
                                scalar1=scl(rot, p, 2), scalar2=scl(rot, p, 3),
                                op0=Alu.mult, op1=Alu.add)
                        else:
                            nc.scalar.activation(
                                out=t[:], in_=shift_view(ero, p),
                                func=Act.Identity, bias=scl(rot, p, 3),
                                scale=scl(rot, p, 2))
                        nc.vector.tensor_tensor(out=acc2[:], in0=t[:],
                                                in1=acc2[:], op=Alu.max)
                    if pending is not None:
                        csum_out(*pending)
                    pending = (rot, acc2)
                csum_out(*pending)
    nc.compile()
    return nc


def _get_program():
    if "nc" not in _CACHE:
        _CACHE["nc"] = _build_program()
    return _CACHE["nc"]


def kernel(x, kernel, timesKernel):
    x = np.ascontiguousarray(np.asarray(x, np.float32))
    Wt = _make_weights(kernel, timesKernel)
    S = _make_csum()

    nc = _get_program()
    from concourse.bass_utils import run_bass_kernel_spmd
    dt_np = np.float16 if USE_FP16 else np.float32
    in_maps = []
    for i in range(NCORES):
        xrh = _build_xrep(x[i * BPC:(i + 1) * BPC]).astype(dt_np)
        in_maps.append({"x8": xrh, "wts": Wt, "csum": S.astype(dt_np)})

    import os
    trace = os.environ.get("BASS_TRACE", "0") == "1"
    r = run_bass_kernel_spmd(nc, in_maps, core_ids=list(range(NCORES)),
                             trace=trace)
    _CACHE["last_results"] = r
    outs = [m["out"] for m in r.results]

    full = np.empty((B, 4, H, W, F), np.float32)
    for i in range(NCORES):
        O = outs[i].astype(np.float32).reshape(4, 40, 26, IL, 128)
        for rot in range(4):
            for f in range(F):
                for j in range(NJ):
                    rws = ROWS[j]
                    for bb in range(BPC):
                        full[i * BPC + bb, rot,
                             CH_START[j]:CH_START[j] + rws, :, f] = \
                            O[rot, f * NJ + j, :rws, bb, :]
    return full
